# revision 82
# baseline (speedup 1.0000x reference)
"""LDA loss (inter/intra hinge) on 8 Trainium2 NeuronCores.

Strategy (data-parallel over B; G=B/16 centers; all-bf16 data path).
All matmul weights span the full 128x128 array -- partial-K/M weights
leave the PE activity monitor 'idle' and the clock gate then pins the
array at 1.2GHz instead of 2.4GHz.

  Host staging: cast path_fea to bf16, rearrange each core's shard to
    p-major [128, b, d] so the device load is fully contiguous.

  Launch 1 (per core, 16384 samples):
    - diff = M*x in a single matmul (M = I - group-mean selector, exact
      in bf16), ACT square, DVE segmented reduce -> d2 [128, 128]
    - centers via a replicated selector matmul (export only; cast to
      bf16 on ACT/DVE, exported per chunk)
    - d2 exported; the tiny per-sample hinge tail runs on host in fp64

  Host: gather centers, build per-core rotated center panels.

  Launch 2 (per core, cyclic-half of the GxG pairwise matrix):
    Uniform triangle: 16 row-chunks of 512; core c owns chunks c and c+8.
    Each row-chunk processes 9 column blocks (its own + next 8 mod 16)
    from a rotated+extended center panel [128, 8704]:
      psum = -2*C_loc^T C_ext  (gram)  +  ones^T sqpanel  (adds ||c_j||^2
        exactly: rows hi, lo, then cancelling +/-hi pairs keep the full
        array busy)  +  512*I on the self-pair diagonal
      zero-screens, one fused op per tile: DVE sum(min(d2+||c_i||^2, 1))
        == count, or ACT sum(relu(1 - d2 - ||c_i||^2)) == 0.
    A screen passes iff no pair in the tile violates the margin (w <= 0
    always, so no cancellation); self-pairs pass via the +512 diagonal.
  Host: all screens pass -> inter = 0 exactly (margin-respecting data);
    any screen fires -> exact numpy fallback (never taken here).
"""
import sys

if "/opt/trn_rl_repo" not in sys.path:
    sys.path.insert(0, "/opt/trn_rl_repo")

import numpy as np
import ml_dtypes

import concourse.bacc as bacc
import concourse.tile as tile
from concourse import mybir
from concourse.bass_utils import run_bass_kernel_spmd

N_CORES = 8
B, D, P = 131072, 128, 16
G = B // P                 # 8192 centers
GL = G // N_CORES          # 1024 local centers (rows) per core
SL = B // N_CORES          # 16384 local samples
NT = SL // 128             # 128 sample tiles / core
CW = 512                   # row-chunk width (16 chunks globally)
EXT = 17 * CW              # 8704 extended column panel
EPS = 1e-3
MI = 0.1

F32 = mybir.dt.float32
BF16 = mybir.dt.bfloat16
AF = mybir.ActivationFunctionType
OP = mybir.AluOpType
AX = mybir.AxisListType

_cache = {}
_last_traces = {}


def _build_launch1():
    nc = bacc.Bacc("TRN2", target_bir_lowering=False, debug=False,
                   num_devices=N_CORES)
    xp = nc.dram_tensor("xp", [128, SL], BF16, kind="ExternalInput").ap()
    sel = nc.dram_tensor("sel", [128, 128], BF16, kind="ExternalInput").ap()
    msub = nc.dram_tensor("msub", [128, 128], BF16, kind="ExternalInput").ap()
    cpack = nc.dram_tensor("cpack", [8, SL], BF16, kind="ExternalOutput").ap()
    d2out = nc.dram_tensor("d2out", [128, NT], F32,
                           kind="ExternalOutput").ap()

    with tile.TileContext(nc) as tc:
        with (
            tc.tile_pool(name="persist", bufs=1) as pp,
            tc.tile_pool(name="work", bufs=3) as wp,
            tc.tile_pool(name="ps1", bufs=2, space="PSUM") as psp,
        ):
            t_sel = pp.tile([128, 128], BF16, tag="sel")
            nc.sync.dma_start(t_sel[:], sel[:])
            t_m = pp.tile([128, 128], BF16, tag="m")
            nc.sync.dma_start(t_m[:], msub[:])
            t_xp = pp.tile([128, SL], BF16, tag="xp")
            for k in range(8):
                nc.sync.dma_start(t_xp[:, 2048 * k:2048 * (k + 1)],
                                  xp[:, 2048 * k:2048 * (k + 1)])
            t_d2 = pp.tile([128, NT], F32, tag="d2")
            t_cseg = pp.tile([8, SL], BF16, tag="cseg")

            # PE warm-up on already-loaded weights
            pwu = psp.tile([128, 2048], F32, tag="ps")
            for _ in range(16):
                nc.tensor.matmul(pwu[:, :128], t_sel[:, :], t_m[:, :],
                                 start=True, stop=True)

            # Interleaved per 2048-col chunk:
            #  centers = sel*x, exported f32 straight from PSUM (host casts;
            #    sel cols replicate the 8 group slots -> full PE activity);
            #  diff = M*x in a single matmul (M = I - group-mean selector),
            #    then square on ACT, segmented reduce over d on DVE
            for k in range(8):
                sl2 = slice(2048 * k, 2048 * (k + 1))
                cps = psp.tile([128, 2048], F32, tag="ps")
                for j in range(4):
                    nc.tensor.matmul(
                        cps[:, 512 * j:512 * (j + 1)], t_sel[:, :],
                        t_xp[:, 2048 * k + 512 * j:2048 * k + 512 * (j + 1)],
                        start=True, stop=True)
                if k % 2 == 0:
                    nc.scalar.copy(t_cseg[:, sl2], cps[:8, :])
                else:
                    nc.vector.tensor_scalar(t_cseg[:, sl2], cps[:8, :], 0.0,
                                            None, op0=OP.add)
                nc.sync.dma_start(cpack[:, sl2], t_cseg[:, sl2])

                dps = psp.tile([128, 2048], F32, tag="ps")
                for j in range(4):
                    nc.tensor.matmul(
                        dps[:, 512 * j:512 * (j + 1)], t_m[:, :],
                        t_xp[:, 2048 * k + 512 * j:2048 * k + 512 * (j + 1)],
                        start=True, stop=True)
                dsq = wp.tile([128, 2048], F32, tag="dsq")
                nc.scalar.activation(dsq[:], dps[:], AF.Square)
                nc.vector.tensor_reduce(
                    t_d2[:, 16 * k:16 * (k + 1)],
                    dsq[:].rearrange("p (b d) -> p b d", d=128),
                    axis=AX.X, op=OP.add)

            for h in range(2):
                slh = slice(64 * h, 64 * (h + 1))
                nc.sync.dma_start(d2out[:, slh], t_d2[:, slh])
    nc.compile()
    return nc


def _build_launch2():
    nc = bacc.Bacc("TRN2", target_bir_lowering=False, debug=False,
                   num_devices=N_CORES)
    ctr = nc.dram_tensor("ctr", [128, EXT], BF16, kind="ExternalInput").ap()
    lh = nc.dram_tensor("lh", [128, GL], BF16, kind="ExternalInput").ap()
    sqrow = nc.dram_tensor("sqrow", [128, EXT], BF16,
                           kind="ExternalInput").ap()
    sqbias2 = nc.dram_tensor("sqbias2", [128, 8], F32,
                             kind="ExternalInput").ap()
    sqbias3 = nc.dram_tensor("sqbias3", [128, 8], F32,
                             kind="ExternalInput").ap()
    ones1 = nc.dram_tensor("ones1", [128, 128], BF16,
                           kind="ExternalInput").ap()
    b512 = nc.dram_tensor("b512", [128, 128], BF16, kind="ExternalInput").ap()
    idI = nc.dram_tensor("idI", [128, 128], BF16, kind="ExternalInput").ap()
    accs = nc.dram_tensor("accs", [128, 40], F32, kind="ExternalOutput").ap()

    with tile.TileContext(nc) as tc:
        with (
            tc.tile_pool(name="persist", bufs=1) as pp,
            tc.tile_pool(name="work", bufs=3) as wp,
            tc.tile_pool(name="ps", bufs=2, space="PSUM") as psp,
        ):
            t_lh = pp.tile([128, GL], BF16, tag="lh")
            nc.sync.dma_start(t_lh[:], lh[:])
            t_ctr = pp.tile([128, EXT], BF16, tag="ctr")
            t_sq = pp.tile([128, EXT], BF16, tag="sq")
            t_o1 = pp.tile([128, 128], BF16, tag="o1")
            t_sb2 = pp.tile([128, 8], F32, tag="sb2")
            t_sb3 = pp.tile([128, 8], F32, tag="sb3")
            nc.sync.dma_start(t_ctr[:, 0:2048], ctr[:, 0:2048])
            nc.sync.dma_start(t_o1[:], ones1[:])
            nc.sync.dma_start(t_sq[:, 0:2048], sqrow[:, 0:2048])
            t_b5 = pp.tile([128, 128], BF16, tag="b5")
            nc.sync.dma_start(t_b5[:], b512[:])
            t_ii = pp.tile([128, 128], BF16, tag="ii")
            nc.sync.dma_start(t_ii[:], idI[:])
            nc.sync.dma_start(t_sb2[:], sqbias2[:])
            nc.sync.dma_start(t_sb3[:], sqbias3[:])
            for k in range(1, 4):
                nc.sync.dma_start(t_ctr[:, 2048 * k:2048 * (k + 1)],
                                  ctr[:, 2048 * k:2048 * (k + 1)])
                nc.sync.dma_start(t_sq[:, 2048 * k:2048 * (k + 1)],
                                  sqrow[:, 2048 * k:2048 * (k + 1)])
            nc.sync.dma_start(t_ctr[:, 8192:EXT], ctr[:, 8192:EXT])
            nc.sync.dma_start(t_sq[:, 8192:EXT], sqrow[:, 8192:EXT])

            t_accs = pp.tile([128, 40], F32, tag="accs")

            # PE warm-up: dense matmul burst on already-loaded lh while ctr
            # still streams in (HAM needs ~4us of sustained PE activity to
            # lift the 1.2GHz cold clock gate)
            pw = psp.tile([128, 1024], F32, tag="pt")
            for _ in range(12):
                nc.tensor.matmul(pw[:, :512], t_lh[:, :128], t_lh[:, :512],
                                 start=True, stop=True)

            # col tiles per m (from base): [0:512) diag block in its own
            # 1-bank psum ring (gram + sq + 512*I self-pair fix), then
            # [512:2048), [2048:3584), [3584:4608) in a [128,1536] ring.
            # Every tile gets ONE fused zero-screen -- DVE:
            # sum(min(d2,1)) == count, or ACT: sum(relu(1-d2)) == 0.
            # Pass iff no pair violates the margin (w <= 0: no
            # cancellation); the far block's double coverage is harmless
            # since screens carry no loss value.  Host falls back to the
            # exact path if any screen fires.
            TILES = [(0, 1024), (1024, 1024), (2048, 1024), (3072, 1024),
                     (4096, 512)]
            for m in range(8):
                base = 0 if m < 4 else 4096
                for t, (c0, cw) in enumerate(TILES):
                    lo = base + c0
                    pt = psp.tile([128, 1024], F32, tag="pt")
                    for j in range((cw + 511) // 512):
                        j0, j1 = 512 * j, min(512 * (j + 1), cw)
                        nc.tensor.matmul(
                            pt[:, j0:j1],
                            t_lh[:, 128 * m:128 * (m + 1)],
                            t_ctr[:, lo + j0:lo + j1],
                            start=True, stop=False)
                    for j in range((cw + 511) // 512):
                        j0, j1 = 512 * j, min(512 * (j + 1), cw)
                        nc.tensor.matmul(
                            pt[:, j0:j1], t_o1[:, :],
                            t_sq[:, lo + j0:lo + j1],
                            start=False, stop=True)
                    ac = t_accs[:, 5 * m + t:5 * m + t + 1]
                    if t == 0:
                        # +512*I on the self-pair diagonal so it passes the
                        # screen (self pairs are not part of the i<j sum)
                        mm = m % 4
                        nc.tensor.matmul(
                            pt[:, 128 * mm:128 * (mm + 1)], t_b5[:, :],
                            t_ii[:, :], start=False, stop=True,
                            skip_group_check=True)
                    ts = wp.tile([128, cw], F32, tag="ts%d" % t)
                    if t in (0, 2, 4):
                        nc.vector.tensor_scalar(
                            ts[:], pt[:, :cw], t_sb2[:, m:m + 1], 1.0,
                            op0=OP.add, op1=OP.min, accum_out=ac)
                    else:
                        nc.scalar.activation(
                            ts[:], pt[:, :cw], AF.Relu,
                            bias=t_sb3[:, m:m + 1], scale=-1.0,
                            accum_out=ac)
            nc.sync.dma_start(accs[:], t_accs[:])
    nc.compile()
    return nc


def _get(name, builder):
    if name not in _cache:
        _cache[name] = builder()
    return _cache[name]


def kernel(path_fea):
    fea = np.asarray(path_fea, dtype=np.float32).reshape(B, D)

    trace = bool(int(__import__("os").environ.get("KERNEL_TRACE", "0")))
    runkw = {}
    if trace:
        try:
            import trace_shim
            trace_shim.install()
            runkw = dict(trace=True)
        except Exception:
            trace = False

    # ---------------- launch 1 ----------------
    nc1 = _get("l1", _build_launch1)
    xbf = fea.astype(ml_dtypes.bfloat16)
    # sel: every output row m holds center (m % 8); negE: -1/16 weight on
    # the 16 replicated rows k with k % 8 == p // 16 -> exactly -c per row.
    sel = np.zeros((128, 128), np.float32)
    for s in range(128):
        for m in range(s // 16, 128, 8):
            sel[s, m] = 1.0 / 16.0
    sel = sel.astype(ml_dtypes.bfloat16)
    # M = I - group-mean: diff = M^T x  in one matmul
    msub = np.zeros((128, 128), np.float32)
    for s in range(128):
        for p in range(128):
            if s // 16 == p // 16:
                msub[s, p] = -1.0 / 16.0
        msub[s, s] += 1.0
    msub = msub.astype(ml_dtypes.bfloat16)
    in1 = []
    for c in range(N_CORES):
        sh = xbf[SL * c:SL * (c + 1)].reshape(NT, 128, D).transpose(1, 0, 2)
        in1.append({"xp": np.ascontiguousarray(sh.reshape(128, SL)),
                    "sel": sel, "msub": msub})
    r1 = run_bass_kernel_spmd(nc1, in1, core_ids=list(range(N_CORES)), **runkw)
    if trace and r1.exec_time_ns is not None:
        print(f"[launch1] HW exec time: {r1.exec_time_ns} ns")
        _last_traces["launch1"] = r1

    # ---------------- host gather ----------------
    centers = np.empty((G, D), ml_dtypes.bfloat16)
    ipart_sum = 0.0
    for c in range(N_CORES):
        cp = r1.results[c]["cpack"].reshape(8, NT, D)      # slot b d
        centers[GL * c:GL * (c + 1)] = cp.transpose(1, 0, 2).reshape(GL, D)
        di = np.sqrt(r1.results[c]["d2out"].astype(np.float64))
        ipart_sum += float((np.maximum(di - MI, 0.0) ** 2).sum())

    cf = centers.astype(np.float32)
    sq = (cf ** 2).sum(1)                                  # [G] f32
    hi = sq.astype(ml_dtypes.bfloat16)
    lo = (sq - hi.astype(np.float32)).astype(ml_dtypes.bfloat16)
    ctrT = np.ascontiguousarray(centers.T)                 # [128, G] bf16
    ones1 = np.ones((128, 128), np.float32).astype(ml_dtypes.bfloat16)
    b512 = (512.0 * np.eye(128, dtype=np.float32)).astype(ml_dtypes.bfloat16)
    idI = np.eye(128, dtype=np.float32).astype(ml_dtypes.bfloat16)

    in2 = []
    rowsets = []
    for c in range(N_CORES):
        idx = (np.arange(EXT) + CW * c) % G
        ctr_ext = np.ascontiguousarray(ctrT[:, idx])
        # sq panel: row0 hi, row1 lo, rows 2..127 alternate +hi/-hi so the
        # ones-weighted column sum stays hi+lo while every PE row toggles
        sqrow = np.empty((128, EXT), ml_dtypes.bfloat16)
        sqrow[0] = hi[idx]
        sqrow[1] = lo[idx]
        sqrow[2::2] = hi[idx]
        neg_hi = (-hi[idx].astype(np.float32)).astype(ml_dtypes.bfloat16)
        sqrow[3::2] = neg_hi
        sqrow = np.ascontiguousarray(sqrow)
        rows = np.concatenate([np.arange(CW * c, CW * (c + 1)),
                               np.arange(CW * (c + 8), CW * (c + 9))])
        rowsets.append(rows)
        lhT = np.ascontiguousarray(
            (cf[rows].T * np.float32(-2.0)).astype(ml_dtypes.bfloat16))
        sqb2 = np.ascontiguousarray(sq[rows].reshape(8, 128).T)
        sqb3 = np.ascontiguousarray(
            (np.float32(1.0) - sq[rows]).reshape(8, 128).T)
        in2.append({"ctr": ctr_ext, "lh": lhT, "sqrow": sqrow,
                    "sqbias2": sqb2, "sqbias3": sqb3, "ones1": ones1,
                    "b512": b512, "idI": idI})

    nc2 = _get("l2", _build_launch2)
    r2 = run_bass_kernel_spmd(nc2, in2, core_ids=list(range(N_CORES)), **runkw)
    if trace and r2.exec_time_ns is not None:
        print(f"[launch2] HW exec time: {r2.exec_time_ns} ns")
        _last_traces["launch2"] = r2

    # ---------------- host reduce ----------------
    # screen columns must equal their column counts (every min(d2,1)==1);
    # otherwise some pair violates the margin -> exact host fallback.
    clean = True
    expect = {0: 1024.0, 1: 0.0, 2: 1024.0, 3: 0.0, 4: 512.0}
    for c in range(N_CORES):
        a = r2.results[c]["accs"].astype(np.float64)       # [128, 40]
        for m in range(8):
            for t, e in expect.items():
                if not np.all(a[:, 5 * m + t] == e):
                    clean = False

    if not clean:
        # margin violations exist: compute inter exactly on host (slow path,
        # never taken for margin-respecting data)
        cd = cf.astype(np.float64)
        sqd = (cd ** 2).sum(1)
        inter_sum = 0.0
        for i0 in range(0, G, 1024):
            blk = sqd[i0:i0 + 1024, None] + sqd[None, :] \
                - 2.0 * (cd[i0:i0 + 1024] @ cd.T)
            dmat = np.sqrt(np.maximum(blk, 0.0))
            h2 = np.maximum(1.0 - dmat, 0.0) ** 2
            iu = np.triu(np.ones((1024, G), dtype=bool), k=1 + i0)
            inter_sum += h2[iu].sum()
    else:
        # all screens certified: no pair violates the margin
        inter_sum = 0.0

    n_pairs = G * (G - 1) / 2.0
    inter = np.float32(inter_sum / n_pairs)
    intra = np.float32(ipart_sum / (G * P))
    return (inter, intra)


# revision 83
# speedup vs baseline: 1.1027x; 1.1027x over previous
"""LDA loss (inter/intra hinge) on 8 Trainium2 NeuronCores.

Strategy (data-parallel over B; G=B/16 centers; all-bf16 data path).
All matmul weights span the full 128x128 array -- partial-K/M weights
leave the PE activity monitor 'idle' and the clock gate then pins the
array at 1.2GHz instead of 2.4GHz.

  Host staging: cast path_fea to bf16, rearrange each core's shard to
    p-major [128, b, d] so the device load is fully contiguous.

  Launch 1 (per core, 16384 samples):
    - diff = M*x in a single matmul (M = I - group-mean selector, exact
      in bf16), ACT square, DVE segmented reduce -> d2 [128, 128]
    - centers via a replicated selector matmul (export only; cast to
      bf16 on ACT/DVE, exported per chunk)
    - d2 exported; the tiny per-sample hinge tail runs on host in fp64

  Host: gather centers, build per-core rotated center panels.

  Launch 2 (per core, cyclic-half of the GxG pairwise matrix):
    Uniform triangle: 16 row-chunks of 512; core c owns chunks c and c+8.
    Each row-chunk processes 9 column blocks (its own + next 8 mod 16)
    from a rotated+extended center panel [128, 8704]:
      psum = -2*C_loc^T C_ext  (gram)  +  ones^T sqpanel  (adds ||c_j||^2
        exactly: rows hi, lo, then cancelling +/-hi pairs keep the full
        array busy)  +  512*I on the self-pair diagonal
      zero-screens, one fused op per tile: DVE sum(min(d2+||c_i||^2, 1))
        == count, or ACT sum(relu(1 - d2 - ||c_i||^2)) == 0.
    A screen passes iff no pair in the tile violates the margin (w <= 0
    always, so no cancellation); self-pairs pass via the +512 diagonal.
  Host: all screens pass -> inter = 0 exactly (margin-respecting data);
    any screen fires -> exact numpy fallback (never taken here).
"""
import sys

if "/opt/trn_rl_repo" not in sys.path:
    sys.path.insert(0, "/opt/trn_rl_repo")

import numpy as np
import ml_dtypes

import concourse.bacc as bacc
import concourse.tile as tile
from concourse import mybir
from concourse.bass_utils import run_bass_kernel_spmd

N_CORES = 8
B, D, P = 131072, 128, 16
G = B // P                 # 8192 centers
GL = G // N_CORES          # 1024 local centers (rows) per core
SL = B // N_CORES          # 16384 local samples
NT = SL // 128             # 128 sample tiles / core
CW = 512                   # row-chunk width (16 chunks globally)
EXT = 17 * CW              # 8704 extended column panel
EPS = 1e-3
MI = 0.1

F32 = mybir.dt.float32
BF16 = mybir.dt.bfloat16
AF = mybir.ActivationFunctionType
OP = mybir.AluOpType
AX = mybir.AxisListType

_cache = {}
_last_traces = {}


def _build_launch1():
    nc = bacc.Bacc("TRN2", target_bir_lowering=False, debug=False,
                   num_devices=N_CORES)
    xp = nc.dram_tensor("xp", [128, SL], BF16, kind="ExternalInput").ap()
    sel = nc.dram_tensor("sel", [128, 128], BF16, kind="ExternalInput").ap()
    msub = nc.dram_tensor("msub", [128, 128], BF16, kind="ExternalInput").ap()
    cpack = nc.dram_tensor("cpack", [8, SL], BF16, kind="ExternalOutput").ap()
    d2out = nc.dram_tensor("d2out", [128, NT], F32,
                           kind="ExternalOutput").ap()

    with tile.TileContext(nc) as tc:
        with (
            tc.tile_pool(name="persist", bufs=1) as pp,
            tc.tile_pool(name="work", bufs=3) as wp,
            tc.tile_pool(name="ps1", bufs=2, space="PSUM") as psp,
        ):
            t_sel = pp.tile([128, 128], BF16, tag="sel")
            nc.sync.dma_start(t_sel[:], sel[:])
            t_m = pp.tile([128, 128], BF16, tag="m")
            nc.sync.dma_start(t_m[:], msub[:])
            t_xp = pp.tile([128, SL], BF16, tag="xp")
            for k in range(8):
                nc.sync.dma_start(t_xp[:, 2048 * k:2048 * (k + 1)],
                                  xp[:, 2048 * k:2048 * (k + 1)])
            t_d2 = pp.tile([128, NT], F32, tag="d2")
            t_cseg = pp.tile([8, SL], BF16, tag="cseg")

            # PE warm-up on already-loaded weights
            pwu = psp.tile([128, 2048], F32, tag="ps")
            for _ in range(16):
                nc.tensor.matmul(pwu[:, :128], t_sel[:, :], t_m[:, :],
                                 start=True, stop=True)

            # Interleaved per 2048-col chunk:
            #  centers = sel*x, exported f32 straight from PSUM (host casts;
            #    sel cols replicate the 8 group slots -> full PE activity);
            #  diff = M*x in a single matmul (M = I - group-mean selector),
            #    then square on ACT, segmented reduce over d on DVE
            for k in range(8):
                sl2 = slice(2048 * k, 2048 * (k + 1))
                cps = psp.tile([128, 2048], F32, tag="ps")
                for j in range(4):
                    nc.tensor.matmul(
                        cps[:, 512 * j:512 * (j + 1)], t_sel[:, :],
                        t_xp[:, 2048 * k + 512 * j:2048 * k + 512 * (j + 1)],
                        start=True, stop=True)
                if k % 2 == 0:
                    nc.scalar.copy(t_cseg[:, sl2], cps[:8, :])
                else:
                    nc.vector.tensor_scalar(t_cseg[:, sl2], cps[:8, :], 0.0,
                                            None, op0=OP.add)
                nc.sync.dma_start(cpack[:, sl2], t_cseg[:, sl2])

                dps = psp.tile([128, 2048], F32, tag="ps")
                for j in range(4):
                    nc.tensor.matmul(
                        dps[:, 512 * j:512 * (j + 1)], t_m[:, :],
                        t_xp[:, 2048 * k + 512 * j:2048 * k + 512 * (j + 1)],
                        start=True, stop=True)
                dsq = wp.tile([128, 2048], F32, tag="dsq")
                nc.scalar.activation(dsq[:], dps[:], AF.Square)
                nc.vector.tensor_reduce(
                    t_d2[:, 16 * k:16 * (k + 1)],
                    dsq[:].rearrange("p (b d) -> p b d", d=128),
                    axis=AX.X, op=OP.add)

            for h in range(2):
                slh = slice(64 * h, 64 * (h + 1))
                nc.sync.dma_start(d2out[:, slh], t_d2[:, slh])
    nc.compile()
    return nc


def _build_launch2():
    nc = bacc.Bacc("TRN2", target_bir_lowering=False, debug=False,
                   num_devices=N_CORES)
    ctr = nc.dram_tensor("ctr", [128, EXT], BF16, kind="ExternalInput").ap()
    lh = nc.dram_tensor("lh", [128, GL], BF16, kind="ExternalInput").ap()
    sqrow = nc.dram_tensor("sqrow", [128, EXT], BF16,
                           kind="ExternalInput").ap()
    sqbias2 = nc.dram_tensor("sqbias2", [128, 8], F32,
                             kind="ExternalInput").ap()
    sqbias3 = nc.dram_tensor("sqbias3", [128, 8], F32,
                             kind="ExternalInput").ap()
    ones1 = nc.dram_tensor("ones1", [128, 128], BF16,
                           kind="ExternalInput").ap()
    b512 = nc.dram_tensor("b512", [128, 128], BF16, kind="ExternalInput").ap()
    idI = nc.dram_tensor("idI", [128, 128], BF16, kind="ExternalInput").ap()
    accs = nc.dram_tensor("accs", [128, 32], F32, kind="ExternalOutput").ap()

    with tile.TileContext(nc) as tc:
        with (
            tc.tile_pool(name="persist", bufs=1) as pp,
            tc.tile_pool(name="work", bufs=3) as wp,
            tc.tile_pool(name="ps", bufs=2, space="PSUM") as psp,
        ):
            t_lh = pp.tile([128, GL], BF16, tag="lh")
            nc.sync.dma_start(t_lh[:], lh[:])
            t_ctr = pp.tile([128, EXT], BF16, tag="ctr")
            t_sq = pp.tile([128, EXT], BF16, tag="sq")
            t_o1 = pp.tile([128, 128], BF16, tag="o1")
            t_sb2 = pp.tile([128, 8], F32, tag="sb2")
            t_sb3 = pp.tile([128, 8], F32, tag="sb3")
            nc.sync.dma_start(t_ctr[:, 0:2048], ctr[:, 0:2048])
            nc.sync.dma_start(t_o1[:], ones1[:])
            nc.sync.dma_start(t_sq[:, 0:2048], sqrow[:, 0:2048])
            t_b5 = pp.tile([128, 128], BF16, tag="b5")
            nc.sync.dma_start(t_b5[:], b512[:])
            t_ii = pp.tile([128, 128], BF16, tag="ii")
            nc.sync.dma_start(t_ii[:], idI[:])
            nc.sync.dma_start(t_sb2[:], sqbias2[:])
            nc.sync.dma_start(t_sb3[:], sqbias3[:])
            for k in range(1, 4):
                nc.sync.dma_start(t_ctr[:, 2048 * k:2048 * (k + 1)],
                                  ctr[:, 2048 * k:2048 * (k + 1)])
                nc.sync.dma_start(t_sq[:, 2048 * k:2048 * (k + 1)],
                                  sqrow[:, 2048 * k:2048 * (k + 1)])
            nc.sync.dma_start(t_ctr[:, 8192:EXT], ctr[:, 8192:EXT])
            nc.sync.dma_start(t_sq[:, 8192:EXT], sqrow[:, 8192:EXT])

            t_accs = pp.tile([128, 32], F32, tag="accs")

            # PE warm-up: dense matmul burst on already-loaded lh while ctr
            # still streams in (HAM needs ~4us of sustained PE activity to
            # lift the 1.2GHz cold clock gate)
            pw = psp.tile([128, 1536], F32, tag="pt")
            for _ in range(12):
                nc.tensor.matmul(pw[:, :512], t_lh[:, :128], t_lh[:, :512],
                                 start=True, stop=True)

            # col tiles per m (from base): [0:512) diag block in its own
            # 1-bank psum ring (gram + sq + 512*I self-pair fix), then
            # [512:2048), [2048:3584), [3584:4608) in a [128,1536] ring.
            # Every tile gets ONE fused zero-screen -- DVE:
            # sum(min(d2,1)) == count, or ACT: sum(relu(1-d2)) == 0.
            # Pass iff no pair violates the margin (w <= 0: no
            # cancellation); the far block's double coverage is harmless
            # since screens carry no loss value.  Host falls back to the
            # exact path if any screen fires.
            TILES = [(0, 512), (512, 1536), (2048, 1536), (3584, 1024)]
            for m in range(8):
                base = 0 if m < 4 else 4096
                for t, (c0, cw) in enumerate(TILES):
                    lo = base + c0
                    if t == 0:
                        pt = psp.tile([128, 512], F32, tag="ptd")
                    else:
                        pt = psp.tile([128, 1536], F32, tag="pt")
                    for j in range((cw + 511) // 512):
                        j0, j1 = 512 * j, min(512 * (j + 1), cw)
                        nc.tensor.matmul(
                            pt[:, j0:j1],
                            t_lh[:, 128 * m:128 * (m + 1)],
                            t_ctr[:, lo + j0:lo + j1],
                            start=True, stop=False)
                    for j in range((cw + 511) // 512):
                        j0, j1 = 512 * j, min(512 * (j + 1), cw)
                        nc.tensor.matmul(
                            pt[:, j0:j1], t_o1[:, :],
                            t_sq[:, lo + j0:lo + j1],
                            start=False, stop=True)
                    ac = t_accs[:, 4 * m + t:4 * m + t + 1]
                    if t == 0:
                        # +512*I on the self-pair diagonal so it passes the
                        # screen (self pairs are not part of the i<j sum)
                        mm = m % 4
                        nc.tensor.matmul(
                            pt[:, 128 * mm:128 * (mm + 1)], t_b5[:, :],
                            t_ii[:, :], start=False, stop=True,
                            skip_group_check=True)
                    ts = wp.tile([128, cw], F32, tag="ts%d" % t)
                    if t in (0, 2):
                        nc.vector.tensor_scalar(
                            ts[:], pt[:, :cw], t_sb2[:, m:m + 1], 1.0,
                            op0=OP.add, op1=OP.min, accum_out=ac)
                    else:
                        nc.scalar.activation(
                            ts[:], pt[:, :cw], AF.Relu,
                            bias=t_sb3[:, m:m + 1], scale=-1.0,
                            accum_out=ac)
            nc.sync.dma_start(accs[:], t_accs[:])
    nc.compile()
    return nc


def _get(name, builder):
    if name not in _cache:
        _cache[name] = builder()
    return _cache[name]


def kernel(path_fea):
    fea = np.asarray(path_fea, dtype=np.float32).reshape(B, D)

    trace = bool(int(__import__("os").environ.get("KERNEL_TRACE", "0")))
    runkw = {}
    if trace:
        try:
            import trace_shim
            trace_shim.install()
            runkw = dict(trace=True)
        except Exception:
            trace = False

    # ---------------- launch 1 ----------------
    nc1 = _get("l1", _build_launch1)
    xbf = fea.astype(ml_dtypes.bfloat16)
    # sel: every output row m holds center (m % 8); negE: -1/16 weight on
    # the 16 replicated rows k with k % 8 == p // 16 -> exactly -c per row.
    sel = np.zeros((128, 128), np.float32)
    for s in range(128):
        for m in range(s // 16, 128, 8):
            sel[s, m] = 1.0 / 16.0
    sel = sel.astype(ml_dtypes.bfloat16)
    # M = I - group-mean: diff = M^T x  in one matmul
    msub = np.zeros((128, 128), np.float32)
    for s in range(128):
        for p in range(128):
            if s // 16 == p // 16:
                msub[s, p] = -1.0 / 16.0
        msub[s, s] += 1.0
    msub = msub.astype(ml_dtypes.bfloat16)
    in1 = []
    for c in range(N_CORES):
        sh = xbf[SL * c:SL * (c + 1)].reshape(NT, 128, D).transpose(1, 0, 2)
        in1.append({"xp": np.ascontiguousarray(sh.reshape(128, SL)),
                    "sel": sel, "msub": msub})
    r1 = run_bass_kernel_spmd(nc1, in1, core_ids=list(range(N_CORES)), **runkw)
    if trace and r1.exec_time_ns is not None:
        print(f"[launch1] HW exec time: {r1.exec_time_ns} ns")
        _last_traces["launch1"] = r1

    # ---------------- host gather ----------------
    centers = np.empty((G, D), ml_dtypes.bfloat16)
    ipart_sum = 0.0
    for c in range(N_CORES):
        cp = r1.results[c]["cpack"].reshape(8, NT, D)      # slot b d
        centers[GL * c:GL * (c + 1)] = cp.transpose(1, 0, 2).reshape(GL, D)
        di = np.sqrt(r1.results[c]["d2out"].astype(np.float64))
        ipart_sum += float((np.maximum(di - MI, 0.0) ** 2).sum())

    cf = centers.astype(np.float32)
    sq = (cf ** 2).sum(1)                                  # [G] f32
    hi = sq.astype(ml_dtypes.bfloat16)
    lo = (sq - hi.astype(np.float32)).astype(ml_dtypes.bfloat16)
    ctrT = np.ascontiguousarray(centers.T)                 # [128, G] bf16
    ones1 = np.ones((128, 128), np.float32).astype(ml_dtypes.bfloat16)
    b512 = (512.0 * np.eye(128, dtype=np.float32)).astype(ml_dtypes.bfloat16)
    idI = np.eye(128, dtype=np.float32).astype(ml_dtypes.bfloat16)

    in2 = []
    rowsets = []
    for c in range(N_CORES):
        idx = (np.arange(EXT) + CW * c) % G
        ctr_ext = np.ascontiguousarray(ctrT[:, idx])
        # sq panel: row0 hi, row1 lo, rows 2..127 alternate +hi/-hi so the
        # ones-weighted column sum stays hi+lo while every PE row toggles
        sqrow = np.empty((128, EXT), ml_dtypes.bfloat16)
        sqrow[0] = hi[idx]
        sqrow[1] = lo[idx]
        sqrow[2::2] = hi[idx]
        neg_hi = (-hi[idx].astype(np.float32)).astype(ml_dtypes.bfloat16)
        sqrow[3::2] = neg_hi
        sqrow = np.ascontiguousarray(sqrow)
        rows = np.concatenate([np.arange(CW * c, CW * (c + 1)),
                               np.arange(CW * (c + 8), CW * (c + 9))])
        rowsets.append(rows)
        lhT = np.ascontiguousarray(
            (cf[rows].T * np.float32(-2.0)).astype(ml_dtypes.bfloat16))
        sqb2 = np.ascontiguousarray(sq[rows].reshape(8, 128).T)
        sqb3 = np.ascontiguousarray(
            (np.float32(1.0) - sq[rows]).reshape(8, 128).T)
        in2.append({"ctr": ctr_ext, "lh": lhT, "sqrow": sqrow,
                    "sqbias2": sqb2, "sqbias3": sqb3, "ones1": ones1,
                    "b512": b512, "idI": idI})

    nc2 = _get("l2", _build_launch2)
    r2 = run_bass_kernel_spmd(nc2, in2, core_ids=list(range(N_CORES)), **runkw)
    if trace and r2.exec_time_ns is not None:
        print(f"[launch2] HW exec time: {r2.exec_time_ns} ns")
        _last_traces["launch2"] = r2

    # ---------------- host reduce ----------------
    # screen columns must equal their column counts (every min(d2,1)==1);
    # otherwise some pair violates the margin -> exact host fallback.
    clean = True
    expect = {0: 512.0, 1: 0.0, 2: 1536.0, 3: 0.0}  # cols 1,3: ACT relu
    for c in range(N_CORES):
        a = r2.results[c]["accs"].astype(np.float64)       # [128, 32]
        for m in range(8):
            for t, e in expect.items():
                if not np.all(a[:, 4 * m + t] == e):
                    clean = False

    if not clean:
        # margin violations exist: compute inter exactly on host (slow path,
        # never taken for margin-respecting data)
        cd = cf.astype(np.float64)
        sqd = (cd ** 2).sum(1)
        inter_sum = 0.0
        for i0 in range(0, G, 1024):
            blk = sqd[i0:i0 + 1024, None] + sqd[None, :] \
                - 2.0 * (cd[i0:i0 + 1024] @ cd.T)
            dmat = np.sqrt(np.maximum(blk, 0.0))
            h2 = np.maximum(1.0 - dmat, 0.0) ** 2
            iu = np.triu(np.ones((1024, G), dtype=bool), k=1 + i0)
            inter_sum += h2[iu].sum()
    else:
        # all screens certified: no pair violates the margin
        inter_sum = 0.0

    n_pairs = G * (G - 1) / 2.0
    inter = np.float32(inter_sum / n_pairs)
    intra = np.float32(ipart_sum / (G * P))
    return (inter, intra)
